# revision 14
# baseline (speedup 1.0000x reference)
"""Trainium2 Bass kernel for nn_Attention_Weighted_Context_Generation.

Computes ctx = A @ F where
  A = weights.reshape(9216, 9216)              (row i = output location)
  F = cnn_feature.reshape(256, 9216).T          [9216, 256]
and returns ctx.reshape(9216, 1, 1, 256) float32.

Sharding: rows of A (the HW/location dim) split across 8 NeuronCores,
1152 rows each; F replicated (per the sharding hint).

Memory-roofline problem (~360-390 GB/s/core HBM) whose bf16 version is
PE-streaming-bound (166k PE cycles @ 2.4 GHz = 69 us/core). To beat
that floor the contraction is split into two precision phases:

 * k-tiles 0..KB-1 in bfloat16 (exact-ish, rel err 2.3e-3 standalone);
 * the rest as fp8-e4m3 DoubleRow super-tiles (256-deep contraction per
   matmul, 2 fp8 MACs/cell/cycle). fp8 alone is too coarse for the
   uniform[0,1) weights, so the device computes B8 @ F8 with
   B = A - 0.5 (mean-split: B is symmetric, halving quantization error)
   and the host adds back 0.5 * colsum(F) exactly. Measured ~1.76e-2
   standalone -> ~1.45e-2 for the hybrid, against a 2e-2 gate.

Both phases keep F as the stationary operand (2 LDWEIGHTS per k-tile
instead of 9; output lands transposed as [C, M] in PSUM, host
transposes back). 6 PSUM chains of [128c, 384m] fp32 accumulate across
both phases; evacuation (DVE cols 0:1152 / ACT 1152:2304) and the two
output stores (sync + ACT HWDGE rings) overlap the kernel tail.

Head optimizations: the first two stream chunks are single k-tiles (the
first matmul waits on a 360 KB DMA, not 720 KB), and 10 junk matmuls
into a scratch PSUM bank warm the PE's HAM clock gate (cold PE runs at
1.2 GHz for its first ~3.4 us) while the first chunk is in flight.
"""

import numpy as np
import ml_dtypes

import concourse.bass as bass
from concourse import mybir
from concourse.bass_utils import run_bass_kernel_spmd

N_CORES = 8
HW = 9216              # number of locations = 96*96
C = 256                # channels
M_PER = HW // N_CORES  # 1152 output rows per core
KT = HW // 128         # 72 contraction tiles
CW = M_PER + C         # 1408 packed columns per k-tile (A^T | F)
# Moving-operand chunks per c-block, PSUM-bank-aligned (512-f32 banks) so
# every chain's start=True bank-clear touches only its own bank: each
# c-block accumulates into its own 3-bank PSUM tensor as 512|512|128.
MJS = [(0, 512), (512, 512), (1024, 128)]
NM = len(MJS)

KB = 8                 # k-tiles computed in bf16
KS = (KT - KB) // 2    # fp8 DoubleRow super-tiles (2 k-tiles each)
K8_OFF = KB * 128      # first contraction row of the fp8 range

NBUF = 6               # SBUF ring depth per phase (chunk slots)
NSEM = 8               # rotation depth for DMA-completion semaphores
NWARM = 10             # junk warm-up matmuls (HAM warm before first chunk)

# bf16 chunk schedule: four 1-k-tile chunks first (smaller first-DMA
# latency before the PE can start, smoother ramp), then 720 KB pairs.
CHUNKS16 = [(0, 1), (1, 1), (2, 1), (3, 1)] + [(k, 2) for k in range(4, KB, 2)]
CHUNKS8 = [(g, 2) for g in range(0, KS, 2)]  # super-tile pairs (720 KB)
N_CH = len(CHUNKS16) + len(CHUNKS8)

F16 = ml_dtypes.bfloat16
F8 = ml_dtypes.float8_e4m3


def build_bass():
    nc = bass.Bass("TRN2", target_bir_lowering=False, debug=False,
                   num_devices=N_CORES)
    atf16 = nc.dram_tensor("atf16", [128, KB * CW], mybir.dt.bfloat16,
                           kind="ExternalInput").ap()
    atf8 = nc.dram_tensor("atf8", [128, KS * 2 * CW], mybir.dt.float8e4,
                          kind="ExternalInput").ap()
    # out[c, m] = ctx[m0+m, c] contribution (pre host correction)
    out = nc.dram_tensor("out", [C, M_PER], mybir.dt.float32,
                         kind="ExternalOutput").ap()

    CWC16 = 2 * CW   # bf16 chunk slot width (elements)
    CWC8 = 4 * CW    # fp8 chunk slot width (2 super-tiles = 4 k-planes)

    from contextlib import ExitStack
    with (
        ExitStack() as stack,
        nc.sbuf_tensor("kb16", [128, NBUF * CWC16], mybir.dt.bfloat16) as kb16,
        nc.sbuf_tensor("kb8", [128, NBUF * CWC8], mybir.dt.float8e4) as kb8,
        nc.sbuf_tensor("junk", [128, 640], mybir.dt.bfloat16) as junk,
        nc.sbuf_tensor("out_sb", [128, 2 * M_PER], mybir.dt.float32) as out_sb,
        nc.psum_tensor("acc0", [128, M_PER], mybir.dt.float32) as acc0,
        nc.psum_tensor("acc1", [128, M_PER], mybir.dt.float32) as acc1,
        nc.psum_tensor("scratch", [128, 512], mybir.dt.float32) as scratch,
        nc.semaphore("mm_sem") as mm_sem,
        nc.semaphore("bank_sem") as bank_sem,
        nc.semaphore("dve_done") as dve_done,
        nc.semaphore("act_done") as act_done,
        nc.semaphore("out_sem") as out_sem,
        nc.Block(no_gpsimd_drain=True) as block,
    ):
        # DMA-completion sems must rotate: a dma_start completes as 16
        # independent per-SDMA-engine increments, and increments of
        # consecutive DMAs interleave across engines. With a single shared
        # sem, "sem >= 16*(ct+1)" does NOT imply DMA ct's data landed.
        # Per-engine descriptor FIFO makes a rotation of NSEM sems safe
        # against up to NSEM-1 DMAs of skew — but only within ONE HWDGE
        # ring, so the two issue rings (sync: bf16, ACT: fp8) get
        # disjoint sem sets.
        dma16_sems = [stack.enter_context(nc.semaphore(f"dma16_sem{i}"))
                      for i in range(len(CHUNKS16))]
        dma8_sems = [stack.enter_context(nc.semaphore(f"dma8_sem{i}"))
                     for i in range(NSEM)]

        @block.sync
        def _(sync):
            # bf16 chunks stream on the sync HWDGE ring; the fp8 stream is
            # issued concurrently from the ACT ring (pre-filling its SBUF
            # ring with spare HBM bandwidth during the head + bf16 phase).
            for ci, (k0, nkt) in enumerate(CHUNKS16):
                if ci >= NBUF:
                    sync.wait_ge(mm_sem, ci - NBUF + 1)
                sync.dma_start(
                    out=kb16[:, (ci % NBUF) * CWC16:
                             (ci % NBUF) * CWC16 + nkt * CW],
                    in_=atf16[:, k0 * CW:(k0 + nkt) * CW],
                ).then_inc(dma16_sems[ci], 16)
            # store c-block 0 (rows 0:128 of out) once DVE evacuated it
            sync.wait_ge(dve_done, 1)
            sync.dma_start(
                out=out[:128, :], in_=out_sb[:, :M_PER],
            ).then_inc(out_sem, 16)
            sync.wait_ge(out_sem, 32)

        @block.tensor
        def _(tensor):
            # HAM warm-up on junk data while the first chunk is in flight
            # (uninitialized SBUF; output confined to a scratch PSUM bank).
            for _ in range(NWARM):
                tensor.matmul(scratch[:, :], junk[:, :128], junk[:, 128:640],
                              start=True, stop=True)

            accs = (acc0, acc1)
            cid = 0
            for k0, nkt in CHUNKS16:
                tensor.wait_ge(dma16_sems[cid], 16)
                b = (cid % NBUF) * CWC16
                inst = None
                for j in range(nkt):
                    base = b + j * CW
                    for cb in range(2):
                        for o, n in MJS:
                            # Each chain owns whole PSUM banks, so its
                            # start=True bank-clear on the first k-tile
                            # cannot clobber any other chain.
                            inst = tensor.matmul(
                                accs[cb][:, o:o + n],
                                kb16[:, base + M_PER + cb * 128:
                                     base + M_PER + (cb + 1) * 128],
                                kb16[:, base + o:base + o + n],
                                start=(cid == 0 and j == 0), stop=False,
                            )
                inst.then_inc(mm_sem, 1)
                cid += 1
            for g0, ng in CHUNKS8:
                fid = cid - len(CHUNKS16)
                tensor.wait_ge(dma8_sems[fid % NSEM], 16 * (fid // NSEM + 1))
                last_chunk = cid == N_CH - 1
                b = (fid % NBUF) * CWC8
                inst = None
                for j in range(ng):
                    # super-tile j of this chunk as [128, 2, 1408]
                    st3d = kb8[:, b + j * 2 * CW:
                               b + (j + 1) * 2 * CW].rearrange(
                                   "p (s w) -> p s w", s=2)
                    fin = last_chunk and j == ng - 1
                    for cb in range(2):
                        for o, n in MJS:
                            inst = tensor.matmul(
                                accs[cb][:, o:o + n],
                                st3d[:, :, M_PER + cb * 128:
                                     M_PER + (cb + 1) * 128],
                                st3d[:, :, o:o + n],
                                start=False, stop=fin,
                                perf_mode=mybir.MatmulPerfMode.DoubleRow,
                            )
                        if fin:
                            # c-block cb finalized: release its evacuation
                            inst.then_inc(bank_sem, 1)
                if not last_chunk:
                    inst.then_inc(mm_sem, 1)
                cid += 1

        @block.vector
        def _(vector):
            vector.wait_ge(bank_sem, 1)
            vector.tensor_copy(out_sb[:, :M_PER],
                               acc0[:, :]).then_inc(dve_done, 1)

        @block.scalar
        def _(scalar):
            # Warm the ACT table early: the first ACTIVATE after boot pays a
            # ~1.4us cold-table cost; a 1-element copy during the stream
            # moves that off the critical tail.
            scalar.copy(out_sb[:1, :1], out_sb[:1, :1])
            # fp8 stream on the ACT HWDGE ring, concurrent with sync's bf16
            # stream — by the time the PE reaches the fp8 phase its ring is
            # full, so the phase runs at the PE DoubleRow floor.
            for fid, (g0, ng) in enumerate(CHUNKS8):
                if fid >= NBUF:
                    scalar.wait_ge(mm_sem,
                                   len(CHUNKS16) + fid - NBUF + 1)
                scalar.dma_start(
                    out=kb8[:, (fid % NBUF) * CWC8:
                            (fid % NBUF) * CWC8 + ng * 2 * CW],
                    in_=atf8[:, g0 * 2 * CW:(g0 + ng) * 2 * CW],
                ).then_inc(dma8_sems[fid % NSEM], 16)
            # c-block 1, stored from ACT's own HWDGE ring, concurrent with
            # sync's store of c-block 0.
            scalar.wait_ge(bank_sem, 2)
            scalar.copy(out_sb[:, M_PER:],
                        acc1[:, :]).then_inc(act_done, 1)
            scalar.wait_ge(act_done, 1)
            scalar.dma_start(
                out=out[128:, :], in_=out_sb[:, M_PER:],
            ).then_inc(out_sem, 16)

    return nc


def prep_inputs(weights: np.ndarray, cnn_feature: np.ndarray):
    """Pack per-core bf16 + fp8 streams (k-major transposed layouts)."""
    A = np.asarray(weights, dtype=np.float32).reshape(HW, HW)
    F = np.ascontiguousarray(
        np.asarray(cnn_feature, dtype=np.float32).reshape(C, HW).T)  # [HW, C]

    A16 = A[:, :K8_OFF].astype(F16)                   # [HW, KB*128]
    B8 = (A[:, K8_OFF:] - np.float32(0.5)).astype(F8)  # [HW, KS*256]
    f16 = F[:K8_OFF].astype(F16).reshape(KB, 128, C).transpose(1, 0, 2)
    f8 = F[K8_OFF:].astype(F8).reshape(KS, 2, 128, C).transpose(2, 0, 1, 3)

    in_maps = []
    for i in range(N_CORES):
        m0 = i * M_PER
        a16 = np.empty((128, KB, CW), dtype=F16)
        a16[:, :, :M_PER] = (A16[m0:m0 + M_PER, :].T
                             .reshape(KB, 128, M_PER).transpose(1, 0, 2))
        a16[:, :, M_PER:] = f16
        a8 = np.empty((128, KS, 2, CW), dtype=F8)
        a8[:, :, :, :M_PER] = (B8[m0:m0 + M_PER, :].T
                               .reshape(KS, 2, 128, M_PER)
                               .transpose(2, 0, 1, 3))
        a8[:, :, :, M_PER:] = f8
        in_maps.append({"atf16": a16.reshape(128, KB * CW),
                        "atf8": a8.reshape(128, KS * 2 * CW)})
    return in_maps


def _host_correction(cnn_feature: np.ndarray) -> np.ndarray:
    """0.5 * colsum of the TRUE F over the fp8 k-range, fp64."""
    F = np.asarray(cnn_feature, dtype=np.float32).reshape(C, HW).T
    return 0.5 * F[K8_OFF:].astype(np.float64).sum(axis=0)  # [C]


def kernel(weights: np.ndarray, cnn_feature: np.ndarray) -> np.ndarray:
    in_maps = prep_inputs(weights, cnn_feature)
    nc = build_bass()
    res = run_bass_kernel_spmd(nc, in_maps, list(range(N_CORES)))
    bias = _host_correction(cnn_feature)  # [C]
    parts = []
    for i in range(N_CORES):
        r = res.results[i]["out"].astype(np.float64)  # [C, M_PER]
        parts.append((r.T + bias[None, :]).astype(np.float32))
    ctx = np.concatenate(parts, axis=0)
    return ctx.reshape(HW, 1, 1, C)
